# revision 28
# baseline (speedup 1.0000x reference)
"""Trainium2 Bass kernel: 7x7 local window attention (ConvNDAttention).

Input  X: [4, 64, 64, 256] fp32 (channel-last).
Output:   [4, 58, 58, 256] fp32.

For each output position (b, r, w): 7x7 input window rows r..r+6, cols
w..w+6; query = center cell (r+3, w+3); keys/values = the other 48 cells.
out = softmax(q . K / 16) @ K.

Sharding: 8 cores = 4 batches x 2 row-halves (30 output rows each, 2-row
overlap).  Per core, 18 tiles of 10x10 queries processed panel-major as
9 sequential pairs; each tile's keys are a 16x16 input patch (256 keys,
2 chunks of 128).

DMA design (measured: ~350 GB/s/core aggregate over 16 engines, ring FIFO
= descriptor order, single sync ring so outputs never preempt inputs):
  xin [128, 400 + 6*2694]  ONE flat input: window-validity mask first,
      then per-panel segments (channel-major panel for scores + spatial-
      major V tiles with ones column), loaded in consumption order.  The
      channel-major panels must duplicate overlapping columns because the
      matmul stationary operand requires one contiguous free dim (BIR
      verifier); V duplication is inherent to patch tiling.  The last
      panel's V rides in 3 per-ri DMAs so the final AVs start earlier.
  out [100, 18, 257] bf16: AV numerator + ones-column denominator; the
      softmax divide happens on the host during gather.  5 output DMAs
      issued after all input issues, so they stream behind the inputs.

Per-pair flow, software-pipelined so the PE never stalls on the exp/mask
chain (PE order: st(0), st(1), av(0), st(2), av(1), ...):
  scores S^T [128k, 2tt, 2j, 100q] (PE, one PSUM bank/pair) ->
  E = exp(S/16) (ACT) -> E *= mask (GPSIMD) ->
  AV [100, 257] per tile into a 2-bank pair tile (PE; ones column gives
  row sums) -> num+den copy to bf16 obuf (DVE) -> store group.

PE warm-up matmuls fed from an on-chip memset constant run right after the
preamble so the HAM clock gate reaches 2.4 GHz before the real matmuls.
"""

import numpy as np
import ml_dtypes

import concourse.bass as bass
import concourse.bacc as bacc
import concourse.mybir as mybir
import concourse.tile as tile

BF16 = ml_dtypes.bfloat16

# ---------------- geometry (hardcoded for X [4,64,64,256]) ----------------
B, H, W, C = 4, 64, 64, 256
HO, WO = H - 6, W - 6          # 58 x 58 output
N_CORES = 8
SH_ROWS_IN = 36                # input rows per shard
SH_ROWS_OUT = 30               # output rows per shard
R0S = [0, 10, 20]              # tile row origins (shard-local output rows)
W0S = [0, 10, 20, 30, 40, 48]  # tile col origins
NPAN = len(W0S)
QT = 10                        # query tile side
KT = 16                        # key patch side
NQ = QT * QT                   # 100 queries per tile
CO = C + 1                     # AV columns (ones column -> denominator)
PAN = SH_ROWS_IN * KT          # 576 panel spatial positions
PSEG = 2 * PAN                 # 1152 channel-major elems per panel
VSEG = 3 * 2 * CO              # 1542 V elems per panel
MSEG = 2 * 2 * NQ + 128        # mask-bias elems + identity (leading)
TOT = MSEG + NPAN * (PSEG + VSEG)   # elems per partition
NT = NPAN * len(R0S)           # 18 output tiles

# Ring layout: each panel's channel-major image travels one slot AHEAD of
# the previous panel's V, so all scores/exp complete while V still streams
# and only the last AV chain trails the final byte.
#   [mask|P0|P1][V0|P2][V1|P3][V2|P4][V3|P5][V4][V5]
_POFS = [MSEG, MSEG + PSEG]
for _t in range(2, NPAN):
    _POFS.append(MSEG + 2 * PSEG + (_t - 2) * (PSEG + VSEG) + VSEG)
_VOFS = [MSEG + 2 * PSEG + _t * (PSEG + VSEG) for _t in range(NPAN - 1)]
_VOFS.append(_VOFS[NPAN - 2] + VSEG)
# input DMA boundaries (elem offsets), in ring order; the V4/V5 tail is
# split at pair-consumption boundaries ([V4r0 V4r1][V4r2 V5r0][V5r1 V5r2])
# so each trailing AV pair starts as soon as its own slice lands
_DMAS = [0, MSEG + 2 * PSEG]
for _t in range(4):
    _DMAS.append(_DMAS[-1] + VSEG + PSEG)
_DMAS += [_DMAS[-1] + 4 * CO, _DMAS[-1] + 8 * CO, TOT]
assert _DMAS[-3] == _VOFS[NPAN - 2] + 4 * CO


def _build_maskbias():
    """[128, 2tt, 2j, 100]: 0 where (key, query) is in-window, -1000
    elsewhere.  Accumulated into the scores PSUM via an identity-weight
    matmul, so exp() alone yields masked E (no post-exp multiply)."""
    m = np.full((2, 128, NQ), -1000.0, dtype=np.float32)
    for j in range(2):
        for p in range(128):
            kh = 8 * j + p // KT
            kw = p % KT
            for q in range(NQ):
                qh, qw = q // QT, q % QT
                dy, dx = kh - qh, kw - qw
                if 0 <= dy <= 6 and 0 <= dx <= 6 and not (dy == 3 and dx == 3):
                    m[j, p, q] = 0.0
    mk1 = np.ascontiguousarray(m.transpose(1, 0, 2))          # [128, 2, 100]
    mk2 = np.broadcast_to(mk1[:, None], (128, 2, 2, NQ))
    return np.ascontiguousarray(mk2).astype(BF16)


_MASKBIAS = _build_maskbias()
_IDENT = np.eye(128, dtype=BF16)

_NC_CACHE = None


def _build_bass():
    global _NC_CACHE
    if _NC_CACHE is not None:
        return _NC_CACHE
    nc = bacc.Bacc("TRN2")
    dt = mybir.dt

    xin = nc.dram_tensor("xin", [128, TOT], dt.bfloat16,
                         kind="ExternalInput")
    out = nc.dram_tensor("out", [NQ, NT, CO], dt.bfloat16,
                         kind="ExternalOutput")

    with tile.TileContext(nc) as tc:
        with (
            tc.tile_pool(name="const", bufs=1) as const_pool,
            tc.tile_pool(name="ework", bufs=4) as e_pool,
            tc.tile_pool(name="ps_s", bufs=3, space="PSUM") as ps_s,
            tc.tile_pool(name="ps_av", bufs=2, space="PSUM") as ps_av,
        ):
            xall = const_pool.tile([128, TOT], dt.bfloat16, tag="xin")
            obuf = const_pool.tile([NQ, NT, CO], dt.bfloat16, tag="obuf")
            wsrc = const_pool.tile([128, 256], dt.bfloat16, tag="wsrc")

            # ring-ordered loads (see _DMAS layout comment)
            for a, b in zip(_DMAS[:-1], _DMAS[1:]):
                nc.sync.dma_start(out=xall[:, a:b], in_=xin[:, a:b])

            mk_sb = xall[:, : 4 * NQ].rearrange("p (a b q) -> p a b q",
                                                a=2, b=2)
            ident = xall[:, 4 * NQ : MSEG]

            # PE warm-up: HAM needs ~3.4us of sustained PE activity to
            # ungate 2.4 GHz; dummies from a memset constant start right
            # after the preamble while inputs stream.
            nc.vector.memset(wsrc[:, :], 0.125)
            # dummy scalar copy: forces the ACT COPY-function table (16KB)
            # to load during the preamble instead of mid-input-stream,
            # where it delays one DMA engine's input lines by ~1us
            tldum = const_pool.tile([128, 1], dt.bfloat16, tag="tldum")
            nc.scalar.copy(tldum[:, :], wsrc[:, 0:1])
            warm = ps_av.tile([128, 2, 512], dt.float32, tag="av")
            for _ in range(22):
                nc.tensor.matmul(warm[:, 0, 0:256], lhsT=wsrc[:, :128],
                                 rhs=wsrc[:, :], start=True, stop=True)

            # E tiles are 128-wide padded slots so the AV LDWEIGHTS gets
            # fast-weight-load (needs NumWeights==128); zero the pads once
            # (the pool cycles these same buffers afterwards).
            for _ in range(4):
                et = e_pool.tile([128, 2, 2, 128], dt.bfloat16, tag="e")
                nc.vector.memset(et[:, :, :, NQ:], 0)

            def panel(ti, k):
                base = _POFS[ti] + k * PAN
                return xall[:, base : base + PAN]

            def vtile(ti, ri, j):
                base = _VOFS[ti] + (ri * 2 + j) * CO
                return xall[:, base : base + CO]

            # Tiles in panel-major order, paired sequentially: only the
            # last ~1.5 pairs depend on the last-arriving panel.
            tiles = [(t, ri, r0) for t in range(NPAN)
                     for ri, r0 in enumerate(R0S)]
            # output DMA group boundaries (in completed pairs); the last
            # two groups are merged so the tail pays one set of per-line
            # packet overheads instead of two
            obnds = [3, 6, 9]

            def scores(s):
                pair = (tiles[2 * s], tiles[2 * s + 1])
                st = ps_s.tile([128, 2, 2, 128], dt.float32, tag="st")
                for tt, (ti, ri, r0) in enumerate(pair):
                    for j in range(2):
                        # mask-bias first (identity weights), then the two
                        # channel-chunk key.query accumulations
                        nc.tensor.matmul(st[:, tt, j, 0:NQ], lhsT=ident,
                                         rhs=mk_sb[:, tt, j, :],
                                         start=True, stop=False)
                        for k in range(2):
                            pan = panel(ti, k)
                            keys = pan[:, (r0 + 8 * j) * KT :
                                        (r0 + 8 * j) * KT + 128]
                            qrys = pan.rearrange(
                                "p (h w) -> p h w", w=KT
                            )[:, r0 + 3 : r0 + 3 + QT, 3 : 3 + QT]
                            nc.tensor.matmul(st[:, tt, j, 0:NQ], lhsT=keys,
                                             rhs=qrys, start=False,
                                             stop=(k == 1))
                return st

            def av_chain(s, st):
                pair = (tiles[2 * s], tiles[2 * s + 1])
                e = e_pool.tile([128, 2, 2, 128], dt.bfloat16, tag="e")
                ev = e[:, :, :, 0:NQ]
                nc.scalar.activation(ev, st[:, :, :, 0:NQ],
                                     mybir.ActivationFunctionType.Exp,
                                     scale=1.0 / 16.0)
                av = ps_av.tile([128, 2, 512], dt.float32, tag="av")
                for tt, (ti, ri, r0) in enumerate(pair):
                    for j in range(2):
                        nc.tensor.matmul(av[:, tt, 0:CO],
                                         lhsT=e[:, tt, j, :],
                                         rhs=vtile(ti, ri, j),
                                         start=(j == 0), stop=(j == 1))
                # num+den to bf16 SBUF in one copy; host divides in gather
                # (last pair's copy on the by-then-idle scalar engine so
                # the two trailing copies run in parallel)
                tl = 2 * s
                if s == 8:
                    nc.scalar.copy(obuf[:, tl : tl + 2, :],
                                   av[0:NQ, :, 0:CO])
                else:
                    nc.vector.tensor_copy(obuf[:, tl : tl + 2, :],
                                          av[0:NQ, :, 0:CO])
                done = s + 1
                if done in obnds:
                    i = obnds.index(done)
                    lo = 2 * (obnds[i - 1] if i else 0)
                    nc.sync.dma_start(out=out[:, lo : 2 * done, :],
                                      in_=obuf[:, lo : 2 * done, :])

            # software pipeline (lag 2): PE order [st0, st1, st2, av0,
            # st3, av1, ...] so scores never wait behind an AV whose E
            # isn't ready yet, even when panels arrive bunched at the end
            pend = []
            for s in range(9):
                st = scores(s)
                if len(pend) == 2:
                    av_chain(*pend.pop(0))
                pend.append((s, st))
            for p in pend:
                av_chain(*p)

    nc.compile()
    _NC_CACHE = nc
    return nc


def _prep_inputs(X):
    X = np.ascontiguousarray(np.asarray(X, dtype=np.float32))
    in_maps = []
    for c in range(N_CORES):
        b, half = c // 2, c % 2
        r_in0 = 0 if half == 0 else H - SH_ROWS_IN          # 0 or 28
        shard = X[b, r_in0 : r_in0 + SH_ROWS_IN]            # [36, 64, 256]
        shard_bf = shard.astype(BF16)
        xin = np.empty((128, TOT), dtype=BF16)
        xin[:, : 4 * NQ] = _MASKBIAS.reshape(128, 4 * NQ)
        xin[:, 4 * NQ : MSEG] = _IDENT
        for t, w0 in enumerate(W0S):
            base = _POFS[t]
            sl = shard_bf[:, w0 : w0 + KT, :]               # [36, 16, 256]
            slT = sl.reshape(PAN, C).T                      # [256, 576]
            xin[:, base : base + PSEG] = slT.reshape(
                2, 128, PAN).transpose(1, 0, 2).reshape(128, PSEG)
            vseg = np.empty((3, 2, 128, CO), dtype=BF16)
            for ri, r0 in enumerate(R0S):
                for j in range(2):
                    patch = shard_bf[r0 + 8 * j : r0 + 8 * j + 8,
                                     w0 : w0 + KT, :]       # [8, 16, 256]
                    vseg[ri, j, :, :C] = patch.reshape(128, C)
                    vseg[ri, j, :, C] = np.asarray(1.0, dtype=BF16)
            vb = _VOFS[t]
            xin[:, vb : vb + VSEG] = vseg.transpose(
                2, 0, 1, 3).reshape(128, VSEG)
        in_maps.append({"xin": np.ascontiguousarray(xin)})
    return in_maps


def _gather_simple(results):
    full = np.empty((B, HO, WO, C), dtype=np.float32)
    ov = 2 * SH_ROWS_OUT - HO                               # overlap rows = 2
    for c in range(N_CORES):
        b, half = c // 2, c % 2
        o = np.asarray(results[c]["out"], dtype=np.float32)
        o = o.reshape(NQ, NT, CO)
        loc = np.empty((SH_ROWS_OUT, WO, C), dtype=np.float32)
        for t, w0 in enumerate(W0S):
            for ri, r0 in enumerate(R0S):
                blk = o[:, t * 3 + ri, :].reshape(QT, QT, CO)
                loc[r0 : r0 + QT, w0 : w0 + QT] = (
                    blk[:, :, :C] / blk[:, :, C:])
        if half == 0:
            full[b, :SH_ROWS_OUT] = loc
        else:
            full[b, SH_ROWS_OUT:] = loc[ov:]
    return full


def _run(X, trace=False, **kw):
    from concourse.bass_utils import run_bass_kernel_spmd

    nc = _build_bass()
    in_maps = _prep_inputs(X)
    res = run_bass_kernel_spmd(nc, in_maps, core_ids=list(range(N_CORES)),
                               trace=trace, **kw)
    return res


def kernel(X):
    res = _run(X, trace=False)
    return _gather_simple(res.results)


# revision 29
# speedup vs baseline: 1.0510x; 1.0510x over previous
"""Trainium2 Bass kernel: 7x7 local window attention (ConvNDAttention).

Input  X: [4, 64, 64, 256] fp32 (channel-last).
Output:   [4, 58, 58, 256] fp32.

For each output position (b, r, w): 7x7 input window rows r..r+6, cols
w..w+6; query = center cell (r+3, w+3); keys/values = the other 48 cells.
out = softmax(q . K / 16) @ K.

Sharding: 8 cores = 4 batches x 2 row-halves (30 output rows each, 2-row
overlap).  Per core, 18 tiles of 10x10 queries processed panel-major as
9 sequential pairs; each tile's keys are a 16x16 input patch (256 keys,
2 chunks of 128).

DMA design (measured: ~350 GB/s/core aggregate over 16 engines, ring FIFO
= descriptor order, single sync ring so outputs never preempt inputs):
  xin [128, 400 + 6*2694]  ONE flat input: window-validity mask first,
      then per-panel segments (channel-major panel for scores + spatial-
      major V tiles with ones column), loaded in consumption order.  The
      channel-major panels must duplicate overlapping columns because the
      matmul stationary operand requires one contiguous free dim (BIR
      verifier); V duplication is inherent to patch tiling.  The last
      panel's V rides in 3 per-ri DMAs so the final AVs start earlier.
  out [100, 18, 257] bf16: AV numerator + ones-column denominator; the
      softmax divide happens on the host during gather.  5 output DMAs
      issued after all input issues, so they stream behind the inputs.

Per-pair flow, software-pipelined so the PE never stalls on the exp/mask
chain (PE order: st(0), st(1), av(0), st(2), av(1), ...):
  scores S^T [128k, 2tt, 2j, 100q] (PE, one PSUM bank/pair) ->
  E = exp(S/16) (ACT) -> E *= mask (GPSIMD) ->
  AV [100, 257] per tile into a 2-bank pair tile (PE; ones column gives
  row sums) -> num+den copy to bf16 obuf (DVE) -> store group.

PE warm-up matmuls fed from an on-chip memset constant run right after the
preamble so the HAM clock gate reaches 2.4 GHz before the real matmuls.
"""

import numpy as np
import ml_dtypes

import concourse.bass as bass
import concourse.bacc as bacc
import concourse.mybir as mybir
import concourse.tile as tile

BF16 = ml_dtypes.bfloat16

# ---------------- geometry (hardcoded for X [4,64,64,256]) ----------------
B, H, W, C = 4, 64, 64, 256
HO, WO = H - 6, W - 6          # 58 x 58 output
N_CORES = 8
SH_ROWS_IN = 36                # input rows per shard
SH_ROWS_OUT = 30               # output rows per shard
R0S = [0, 10, 20]              # tile row origins (shard-local output rows)
W0S = [0, 10, 20, 30, 40, 48]  # tile col origins
NPAN = len(W0S)
QT = 10                        # query tile side
KT = 16                        # key patch side
NQ = QT * QT                   # 100 queries per tile
CO = C + 1                     # AV columns (ones column -> denominator)
PAN = SH_ROWS_IN * KT          # 576 panel spatial positions
PSEG = 2 * PAN                 # 1152 channel-major elems per panel
VSEG = 3 * 2 * CO              # 1542 V elems per panel
MSEG = 2 * 2 * NQ + 128        # mask-bias elems + identity (leading)
TOT = MSEG + NPAN * (PSEG + VSEG)   # elems per partition
NT = NPAN * len(R0S)           # 18 output tiles

# Ring layout: each panel's channel-major image travels one slot AHEAD of
# the previous panel's V, so all scores/exp complete while V still streams
# and only the last AV chain trails the final byte.
#   [mask|P0|P1][V0|P2][V1|P3][V2|P4][V3|P5][V4][V5]
_POFS = [MSEG, MSEG + PSEG]
for _t in range(2, NPAN):
    _POFS.append(MSEG + 2 * PSEG + (_t - 2) * (PSEG + VSEG) + VSEG)
_VOFS = [MSEG + 2 * PSEG + _t * (PSEG + VSEG) for _t in range(NPAN - 1)]
_VOFS.append(_VOFS[NPAN - 2] + VSEG)
# input DMA boundaries (elem offsets), in ring order; the V4/V5 tail is
# split at pair-consumption boundaries ([V4r0 V4r1][V4r2 V5r0][V5r1 V5r2])
# so each trailing AV pair starts as soon as its own slice lands
_DMAS = [0, MSEG + 2 * PSEG]
for _t in range(4):
    _DMAS.append(_DMAS[-1] + VSEG + PSEG)
_DMAS += [_DMAS[-1] + 4 * CO, _DMAS[-1] + 8 * CO, TOT]
assert _DMAS[-3] == _VOFS[NPAN - 2] + 4 * CO


def _build_maskbias():
    """[128, 2tt, 2j, 100]: 0 where (key, query) is in-window, -1000
    elsewhere.  Accumulated into the scores PSUM via an identity-weight
    matmul, so exp() alone yields masked E (no post-exp multiply)."""
    m = np.full((2, 128, NQ), -1000.0, dtype=np.float32)
    for j in range(2):
        for p in range(128):
            kh = 8 * j + p // KT
            kw = p % KT
            for q in range(NQ):
                qh, qw = q // QT, q % QT
                dy, dx = kh - qh, kw - qw
                if 0 <= dy <= 6 and 0 <= dx <= 6 and not (dy == 3 and dx == 3):
                    m[j, p, q] = 0.0
    mk1 = np.ascontiguousarray(m.transpose(1, 0, 2))          # [128, 2, 100]
    mk2 = np.broadcast_to(mk1[:, None], (128, 2, 2, NQ))
    return np.ascontiguousarray(mk2).astype(BF16)


_MASKBIAS = _build_maskbias()
_IDENT = np.eye(128, dtype=BF16)

_NC_CACHE = None


def _build_bass():
    global _NC_CACHE
    if _NC_CACHE is not None:
        return _NC_CACHE
    nc = bacc.Bacc("TRN2")
    dt = mybir.dt

    xin = nc.dram_tensor("xin", [128, TOT], dt.bfloat16,
                         kind="ExternalInput")
    out = nc.dram_tensor("out", [NQ, NT, CO], dt.bfloat16,
                         kind="ExternalOutput")

    with tile.TileContext(nc) as tc:
        with (
            tc.tile_pool(name="const", bufs=1) as const_pool,
            tc.tile_pool(name="ework", bufs=4) as e_pool,
            tc.tile_pool(name="ps_s", bufs=4, space="PSUM") as ps_s,
            tc.tile_pool(name="ps_av", bufs=2, space="PSUM") as ps_av,
        ):
            xall = const_pool.tile([128, TOT], dt.bfloat16, tag="xin")
            obuf = const_pool.tile([NQ, NT, CO], dt.bfloat16, tag="obuf")
            wsrc = const_pool.tile([128, 256], dt.bfloat16, tag="wsrc")

            # ring-ordered loads (see _DMAS layout comment)
            for a, b in zip(_DMAS[:-1], _DMAS[1:]):
                nc.sync.dma_start(out=xall[:, a:b], in_=xin[:, a:b])

            mk_sb = xall[:, : 4 * NQ].rearrange("p (a b q) -> p a b q",
                                                a=2, b=2)
            ident = xall[:, 4 * NQ : MSEG]

            # PE warm-up: HAM needs ~3.4us of sustained PE activity to
            # ungate 2.4 GHz; dummies from a memset constant start right
            # after the preamble while inputs stream.
            nc.vector.memset(wsrc[:, :], 0.125)
            warm = ps_av.tile([128, 2, 512], dt.float32, tag="av")
            for _ in range(22):
                nc.tensor.matmul(warm[:, 0, 0:256], lhsT=wsrc[:, :128],
                                 rhs=wsrc[:, :], start=True, stop=True)

            # E tiles are 128-wide padded slots so the AV LDWEIGHTS gets
            # fast-weight-load (needs NumWeights==128); zero the pads once
            # (the pool cycles these same buffers afterwards).
            for _ in range(4):
                et = e_pool.tile([128, 2, 2, 128], dt.bfloat16, tag="e")
                nc.vector.memset(et[:, :, :, NQ:], 0)

            def panel(ti, k):
                base = _POFS[ti] + k * PAN
                return xall[:, base : base + PAN]

            def vtile(ti, ri, j):
                base = _VOFS[ti] + (ri * 2 + j) * CO
                return xall[:, base : base + CO]

            # Tiles in panel-major order, paired sequentially: only the
            # last ~1.5 pairs depend on the last-arriving panel.
            tiles = [(t, ri, r0) for t in range(NPAN)
                     for ri, r0 in enumerate(R0S)]
            # output DMA group boundaries (in completed pairs)
            obnds = [3, 6, 8, 9]

            def scores(s):
                pair = (tiles[2 * s], tiles[2 * s + 1])
                st = ps_s.tile([128, 2, 2, 128], dt.float32, tag="st")
                for tt, (ti, ri, r0) in enumerate(pair):
                    for j in range(2):
                        # mask-bias first (identity weights), then the two
                        # channel-chunk key.query accumulations
                        nc.tensor.matmul(st[:, tt, j, 0:NQ], lhsT=ident,
                                         rhs=mk_sb[:, tt, j, :],
                                         start=True, stop=False)
                        for k in range(2):
                            pan = panel(ti, k)
                            keys = pan[:, (r0 + 8 * j) * KT :
                                        (r0 + 8 * j) * KT + 128]
                            qrys = pan.rearrange(
                                "p (h w) -> p h w", w=KT
                            )[:, r0 + 3 : r0 + 3 + QT, 3 : 3 + QT]
                            nc.tensor.matmul(st[:, tt, j, 0:NQ], lhsT=keys,
                                             rhs=qrys, start=False,
                                             stop=(k == 1))
                return st

            def av_chain(s, st):
                pair = (tiles[2 * s], tiles[2 * s + 1])
                e = e_pool.tile([128, 2, 2, 128], dt.bfloat16, tag="e")
                ev = e[:, :, :, 0:NQ]
                nc.scalar.activation(ev, st[:, :, :, 0:NQ],
                                     mybir.ActivationFunctionType.Exp,
                                     scale=1.0 / 16.0)
                av = ps_av.tile([128, 2, 512], dt.float32, tag="av")
                for tt, (ti, ri, r0) in enumerate(pair):
                    for j in range(2):
                        nc.tensor.matmul(av[:, tt, 0:CO],
                                         lhsT=e[:, tt, j, :],
                                         rhs=vtile(ti, ri, j),
                                         start=(j == 0), stop=(j == 1))
                # num+den to bf16 SBUF in one copy; host divides in gather
                # (last pair's copy on the by-then-idle scalar engine so
                # the two trailing copies run in parallel)
                tl = 2 * s
                if s == 8:
                    nc.scalar.copy(obuf[:, tl : tl + 2, :],
                                   av[0:NQ, :, 0:CO])
                else:
                    nc.vector.tensor_copy(obuf[:, tl : tl + 2, :],
                                          av[0:NQ, :, 0:CO])
                done = s + 1
                if done in obnds:
                    i = obnds.index(done)
                    lo = 2 * (obnds[i - 1] if i else 0)
                    nc.sync.dma_start(out=out[:, lo : 2 * done, :],
                                      in_=obuf[:, lo : 2 * done, :])

            # software pipeline (lag 3): PE order [st0..st3, av0, st4,
            # av1, ...] so all scores (and exps) complete before the
            # last V slices land; the trailing AVs are then purely
            # DMA-gated
            pend = []
            for s in range(9):
                st = scores(s)
                if len(pend) == 3:
                    av_chain(*pend.pop(0))
                pend.append((s, st))
            for p in pend:
                av_chain(*p)

    nc.compile()
    _NC_CACHE = nc
    return nc


def _prep_inputs(X):
    X = np.ascontiguousarray(np.asarray(X, dtype=np.float32))
    in_maps = []
    for c in range(N_CORES):
        b, half = c // 2, c % 2
        r_in0 = 0 if half == 0 else H - SH_ROWS_IN          # 0 or 28
        shard = X[b, r_in0 : r_in0 + SH_ROWS_IN]            # [36, 64, 256]
        shard_bf = shard.astype(BF16)
        xin = np.empty((128, TOT), dtype=BF16)
        xin[:, : 4 * NQ] = _MASKBIAS.reshape(128, 4 * NQ)
        xin[:, 4 * NQ : MSEG] = _IDENT
        for t, w0 in enumerate(W0S):
            base = _POFS[t]
            sl = shard_bf[:, w0 : w0 + KT, :]               # [36, 16, 256]
            slT = sl.reshape(PAN, C).T                      # [256, 576]
            xin[:, base : base + PSEG] = slT.reshape(
                2, 128, PAN).transpose(1, 0, 2).reshape(128, PSEG)
            vseg = np.empty((3, 2, 128, CO), dtype=BF16)
            for ri, r0 in enumerate(R0S):
                for j in range(2):
                    patch = shard_bf[r0 + 8 * j : r0 + 8 * j + 8,
                                     w0 : w0 + KT, :]       # [8, 16, 256]
                    vseg[ri, j, :, :C] = patch.reshape(128, C)
                    vseg[ri, j, :, C] = np.asarray(1.0, dtype=BF16)
            vb = _VOFS[t]
            xin[:, vb : vb + VSEG] = vseg.transpose(
                2, 0, 1, 3).reshape(128, VSEG)
        in_maps.append({"xin": np.ascontiguousarray(xin)})
    return in_maps


def _gather_simple(results):
    full = np.empty((B, HO, WO, C), dtype=np.float32)
    ov = 2 * SH_ROWS_OUT - HO                               # overlap rows = 2
    for c in range(N_CORES):
        b, half = c // 2, c % 2
        o = np.asarray(results[c]["out"], dtype=np.float32)
        o = o.reshape(NQ, NT, CO)
        loc = np.empty((SH_ROWS_OUT, WO, C), dtype=np.float32)
        for t, w0 in enumerate(W0S):
            for ri, r0 in enumerate(R0S):
                blk = o[:, t * 3 + ri, :].reshape(QT, QT, CO)
                loc[r0 : r0 + QT, w0 : w0 + QT] = (
                    blk[:, :, :C] / blk[:, :, C:])
        if half == 0:
            full[b, :SH_ROWS_OUT] = loc
        else:
            full[b, SH_ROWS_OUT:] = loc[ov:]
    return full


def _run(X, trace=False, **kw):
    from concourse.bass_utils import run_bass_kernel_spmd

    nc = _build_bass()
    in_maps = _prep_inputs(X)
    res = run_bass_kernel_spmd(nc, in_maps, core_ids=list(range(N_CORES)),
                               trace=trace, **kw)
    return res


def kernel(X):
    res = _run(X, trace=False)
    return _gather_simple(res.results)
